# revision 15
# baseline (speedup 1.0000x reference)
"""CapsuleLayer dynamic-routing kernel for 8 Trainium2 NeuronCores.

Algorithm (validated vs reference in numpy):
  priors P[c,b,n,o] = sum_i x[b,n,i] W[c,n,i,o]; logits are constant along o,
  so routing state is L[c,b,n]. Per routing iteration:
    probs = exp(L)/denom       (softmax over n; no max-subtraction: |L| < ~30)
    s[c,b,o] = sum_n probs*P = (1/denom) sum_{(n,i)} (x*exp(L)) W   <- matmul
    v = squash(s) = s_u * g,  g = nrm/((1+nrm)*sqrt(nrm+eps))/denom
    a[c,b,n] = sum_o P*v = sum_i x * (W^T v)       <- matmul + blockdiag reduce
    L += a
  Sharding: N=1152 split 8 ways (144 route nodes/core); one 174KB AllReduce of
  s-partials + softmax denominators per iteration. Every core computes the
  identical full output; core 0's is returned.
"""

import sys

sys.path.insert(0, "/opt/trn_rl_repo")

import numpy as np
import ml_dtypes

import concourse.bass as bass
import concourse.bacc as bacc
import concourse.mybir as mybir
from concourse import bass_utils
from concourse.tile import TileContext

BF16 = mybir.dt.bfloat16
F32 = mybir.dt.float32
AF = mybir.ActivationFunctionType
ALU = mybir.AluOpType

B, N, CI, C, CO = 256, 1152, 8, 10, 16
NCORES = 8
NLOC = N // NCORES          # 144 route nodes per core
K = NLOC * CI               # 1152 local contraction length (n,i)
NCH = K // 128              # 9 partition chunks of (n,i)
NFULL = 128 // CI           # 16 n per chunk
EPS = 1e-8
NITER = 3
CB = C * B                  # 2560


def _build_blockdiag() -> np.ndarray:
    """a-reduce lhsT constants: cols 0..1023 hold 8 [128,128] blocks (chunk j
    maps (n16,i8) row q -> out partition 16j + q//8); cols 1024..1039 hold the
    9th chunk's [128,16] block (out partition q//8)."""
    blk = np.zeros((128, 8 * 128 + 16), np.float32)
    for j in range(8):
        for q in range(128):
            blk[q, 128 * j + 16 * j + q // CI] = 1.0
    for q in range(128):
        blk[q, 1024 + q // CI] = 1.0
    return blk


def _bcast_ap(ap, dim_idx, count):
    """Insert a stride-0 (broadcast) dim into an AP at position dim_idx."""
    dims = [list(d) for d in ap.ap]
    dims.insert(dim_idx, [0, count])
    return bass.AP(tensor=ap.tensor, offset=ap.offset, ap=dims)


def _reshaped_ap(ap, dims):
    return bass.AP(tensor=ap.tensor, offset=ap.offset, ap=[list(d) for d in dims])


def build_kernel():
    nc = bacc.Bacc("TRN2", target_bir_lowering=False, debug=False,
                   num_devices=NCORES)
    xT_d = nc.dram_tensor("xT", [K, B], BF16, kind="ExternalInput")
    xTf_d = nc.dram_tensor("xTf", [K, B], F32, kind="ExternalInput")
    w1_d = nc.dram_tensor("w1", [C, K, CO], BF16, kind="ExternalInput")
    w2_d = nc.dram_tensor("w2", [C, CO, K], F32, kind="ExternalInput")
    blk_d = nc.dram_tensor("blk", [128, 1040], F32, kind="ExternalInput")
    vout_d = nc.dram_tensor("vout", [CO, CB], F32, kind="ExternalOutput")

    with TileContext(nc) as tc:
        _emit(tc, xT_d.ap(), xTf_d.ap(), w1_d.ap(), w2_d.ap(), blk_d.ap(), vout_d.ap())
    nc.compile()
    return nc


def _emit(tc, xT_d, xTf_d, w1_d, w2_d, blk_d, vout_d):
    from contextlib import ExitStack
    with ExitStack() as ctx:
        _emit_body(ctx, tc, xT_d, xTf_d, w1_d, w2_d, blk_d, vout_d)


def _emit_body(ctx, tc, xT_d, xTf_d, w1_d, w2_d, blk_d, vout_d):
    nc = tc.nc
    state = ctx.enter_context(tc.tile_pool(name="state", bufs=1))
    erep_p = ctx.enter_context(tc.tile_pool(name="erep", bufs=2))
    z_p = ctx.enter_context(tc.tile_pool(name="zp", bufs=1))
    gtmp_p = ctx.enter_context(tc.tile_pool(name="gtmp", bufs=2))
    dram = ctx.enter_context(tc.tile_pool(name="dram", bufs=2, space="DRAM"))
    ups_p = ctx.enter_context(tc.tile_pool(name="ups", bufs=1, space="PSUM"))
    acc_p = ctx.enter_context(tc.tile_pool(name="acc", bufs=2, space="PSUM"))
    tiny_p = ctx.enter_context(tc.tile_pool(name="tinyps", bufs=1, space="PSUM"))

    # ---- persistent SBUF state ----
    xT = state.tile([128, NCH * B], BF16)        # [(n,i) chunk-part, (j, b)]
    xTf = state.tile([128, NCH * B], F32)        # fp32 copy for agreement
    w1 = state.tile([128, C * NCH * CO], BF16)   # s-matmul lhsT blocks
    w2 = state.tile([16, C * K], F32)            # U-matmul lhsT blocks
    blk = state.tile([128, 1040], F32)           # a-reduce lhsT blocks
    ones128 = state.tile([128, 1], BF16)
    ones16f = state.tile([16, 1], F32)
    L = state.tile([128, CB], F32)               # logits, partition = local n
    L9 = state.tile([16, CB], F32)               # local n in [128,144)
    expL = state.tile([128, CB], BF16)
    expL9 = state.tile([16, CB], BF16)
    y_all = state.tile([128, C * NCH * B], BF16)  # y = x*expL per c
    s_u = state.tile([16, CB], F32)              # AllReduced s_unnorm [o,(c,b)]
    s2 = state.tile([16, CB], F32)
    v_f = state.tile([16, CB], F32)
    g_rep = state.tile([16, CB], F32)
    den_g = state.tile([128, 20], F32)
    ssq_g = state.tile([128, 20], F32)

    # ---- load inputs / init state ----
    for j in range(NCH):
        nc.sync.dma_start(out=xT[:, j * B:(j + 1) * B],
                          in_=xT_d[j * 128:(j + 1) * 128, :])
        nc.sync.dma_start(out=xTf[:, j * B:(j + 1) * B],
                          in_=xTf_d[j * 128:(j + 1) * 128, :])
    for c in range(C):
        # w1[c] chunk j of 128 (n,i)-rows -> w1 cols (c*NCH+j)*CO .. +CO
        src = w1_d[c].rearrange("(j p) o -> p j o", j=NCH)
        dst = w1[:, c * NCH * CO:(c + 1) * NCH * CO].rearrange(
            "p (j o) -> p j o", j=NCH)
        nc.sync.dma_start(out=dst, in_=src)
    nc.sync.dma_start(out=w2[:].rearrange("p (c k) -> p c k", c=C),
                      in_=w2_d.rearrange("c o k -> o c k"))
    nc.sync.dma_start(out=blk[:], in_=blk_d[:, :])
    eps128 = state.tile([128, 1], F32)
    nc.vector.memset(eps128[:], EPS)
    nc.vector.memset(ones128[:], 1.0)
    nc.vector.memset(ones16f[:], 1.0)
    nc.vector.memset(L[:], 0.0)
    nc.vector.memset(L9[:], 0.0)
    nc.vector.memset(expL[:], 1.0)   # exp(0)
    nc.vector.memset(expL9[:], 1.0)

    for it in range(NITER):
        blob_in = dram.tile([170, B], F32, tag="blob_in")
        blob_out = dram.tile([170, B], F32, tag="blob_out")

        # ---- s-phase: s_unnorm partial [16=o, 256=b] per c ----
        for c in range(C):
            s_ps = acc_p.tile([16, B], F32, tag="acc")
            for j in range(NCH):
                rhs = (xT[:, j * B:(j + 1) * B] if it == 0 else
                       y_all[:, (c * NCH + j) * B:(c * NCH + j + 1) * B])
                lo = (c * NCH + j) * CO
                nc.tensor.matmul(s_ps[:], w1[:, lo:lo + CO], rhs,
                                 start=(j == 0), stop=(j == NCH - 1))
            nc.scalar.copy(s2[:, c * B:(c + 1) * B], s_ps[:])
            nc.sync.dma_start(out=blob_in[16 * c:16 * c + 16, :],
                              in_=s2[:, c * B:(c + 1) * B])

        # ---- softmax denominator partials: sum_n expL ----
        for c in range(C):
            den_ps = tiny_p.tile([1, B], F32, tag="tiny")
            nc.tensor.matmul(den_ps[:], ones128[:],
                             expL[:, c * B:(c + 1) * B],
                             start=True, stop=False)
            nc.tensor.matmul(den_ps[:], ones128[0:16, :],
                             expL9[:, c * B:(c + 1) * B],
                             start=False, stop=True)
            nc.scalar.copy(g_rep[0:1, c * B:(c + 1) * B], den_ps[:])
        nc.sync.dma_start(out=blob_in[160:170, :], in_=g_rep[0:1, :])

        nc.gpsimd.collective_compute(
            "AllReduce", ALU.add,
            replica_groups=[list(range(NCORES))],
            ins=[blob_in.opt()], outs=[blob_out.opt()],
        )

        # s_u [o=16, (c,b)] <- blob_out rows [16c..16c+16); den -> [128, 20]
        nc.sync.dma_start(out=s_u[:].rearrange("o (c b) -> o c b", c=C),
                          in_=blob_out[0:160, :].rearrange(
                              "(c o) b -> o c b", c=C))
        src = _reshaped_ap(blob_out[160:170, :], [[20, 128], [1, 20]])
        nc.sync.dma_start(out=den_g[:], in_=src)

        # ---- squash: v = s_u * g,  g = nrm/((1+nrm)sqrt(nrm+eps))/denom ----
        nc.vector.tensor_mul(s2[:], s_u[:], s_u[:])
        ssq_stage = dram.tile([CB], F32, tag="ssq_stage")
        for t in range(5):
            ssq_ps = tiny_p.tile([1, 512], F32, tag="tiny")
            nc.tensor.matmul(ssq_ps[:], ones16f[:], s2[:, 512 * t:512 * (t + 1)],
                             start=True, stop=True)
            nc.scalar.copy(v_f[0:1, 512 * t:512 * (t + 1)], ssq_ps[:])
        nc.sync.dma_start(out=ssq_stage[:], in_=v_f[0:1, :])
        src = _reshaped_ap(ssq_stage, [[20, 128], [1, 20]])
        nc.sync.dma_start(out=ssq_g[:], in_=src)

        rD = gtmp_p.tile([128, 20], F32, tag="g0")
        nrm = gtmp_p.tile([128, 20], F32, tag="g1")
        t1 = gtmp_p.tile([128, 20], F32, tag="g2")
        t2 = gtmp_p.tile([128, 20], F32, tag="g3")
        g = gtmp_p.tile([128, 20], F32, tag="g4")
        nc.vector.reciprocal(rD[:], den_g[:])
        nc.vector.tensor_mul(t1[:], ssq_g[:], rD[:])
        nc.vector.tensor_mul(nrm[:], t1[:], rD[:])
        nc.scalar.activation(t1[:], nrm[:], AF.Sqrt, bias=eps128[:])  # sqrt(nrm+eps)
        nc.vector.tensor_scalar_add(t2[:], nrm[:], 1.0)         # 1+nrm
        nc.vector.tensor_mul(t2[:], t2[:], t1[:])
        nc.vector.tensor_mul(t2[:], t2[:], den_g[:])
        nc.vector.reciprocal(t2[:], t2[:])
        nc.vector.tensor_mul(g[:], nrm[:], t2[:])

        g_stage = dram.tile([CB], F32, tag="g_stage")
        dst = _reshaped_ap(g_stage, [[20, 128], [1, 20]])
        nc.sync.dma_start(out=dst, in_=g[:])
        src = _reshaped_ap(g_stage, [[0, 16], [1, CB]])
        nc.gpsimd.dma_start(out=g_rep[:], in_=src)
        nc.vector.tensor_mul(v_f[:], s_u[:], g_rep[:])

        if it == NITER - 1:
            nc.sync.dma_start(out=vout_d[:, :], in_=v_f[:])
            break

        # ---- agreement: a[c,b,n] = sum_o P*v; L += a ----
        for c in range(C):
            u_ps = ups_p.tile([128, NCH * B], F32, tag="ups")
            for j in range(NCH):
                lo = c * K + 128 * j
                nc.tensor.matmul(u_ps[:, j * B:(j + 1) * B],
                                 w2[:, lo:lo + 128],
                                 v_f[:, c * B:(c + 1) * B],
                                 start=True, stop=True)
            z = z_p.tile([128, NCH * B], F32, tag="z")
            nc.vector.tensor_mul(z[:], xTf[:], u_ps[:])
            a_ps = acc_p.tile([128, B], F32, tag="acc")
            a9_ps = acc_p.tile([16, B], F32, tag="acc")
            for j in range(8):
                nc.tensor.matmul(a_ps[:], blk[:, 128 * j:128 * (j + 1)],
                                 z[:, j * B:(j + 1) * B],
                                 start=(j == 0), stop=(j == 7))
            nc.tensor.matmul(a9_ps[:], blk[:, 1024:1040],
                             z[:, 8 * B:9 * B], start=True, stop=True)
            nc.vector.tensor_add(L[:, c * B:(c + 1) * B],
                                 L[:, c * B:(c + 1) * B], a_ps[:])
            nc.vector.tensor_add(L9[:, c * B:(c + 1) * B],
                                 L9[:, c * B:(c + 1) * B], a9_ps[:])
        nc.scalar.activation(expL[:], L[:], AF.Exp)
        nc.scalar.activation(expL9[:], L9[:], AF.Exp)

        # spill expL to DRAM as [c, n_local(144), b], broadcast-read back into
        # y layout [(n16,i8), b] per (c, chunk), multiply by x.
        spill = dram.tile([C, NLOC, B], BF16, tag="spill")
        for c in range(C):
            nc.sync.dma_start(out=spill[c, 0:128, :],
                              in_=expL[:, c * B:(c + 1) * B])
            nc.sync.dma_start(out=spill[c, 128:NLOC, :],
                              in_=expL9[:, c * B:(c + 1) * B])
        for c in range(C):
            erep = erep_p.tile([128, NCH * B], BF16, tag="erep")
            for j in range(NCH):
                src = _bcast_ap(spill[c, 16 * j:16 * j + NFULL, :], 1, CI)
                nc.gpsimd.dma_start(out=erep[:, j * B:(j + 1) * B], in_=src)
            nc.vector.tensor_mul(y_all[:, c * NCH * B:(c + 1) * NCH * B],
                                 xT[:], erep[:])


def _prep_inputs(x: np.ndarray, route_weights: np.ndarray):
    """Host-side sharding + layout prep. Returns per-core input maps."""
    bf = ml_dtypes.bfloat16
    blk = _build_blockdiag()
    in_maps = []
    for k in range(NCORES):
        sl = slice(k * NLOC, (k + 1) * NLOC)
        xT = np.ascontiguousarray(
            x[:, sl, :].transpose(1, 2, 0).reshape(K, B)).astype(bf)
        w1 = np.ascontiguousarray(
            route_weights[:, sl].reshape(C, K, CO)).astype(bf)
        w1f = np.ascontiguousarray(
            route_weights[:, sl].reshape(C, K, CO)).astype(np.float32)
        w2 = np.ascontiguousarray(w1f.transpose(0, 2, 1))
        xTf = np.ascontiguousarray(
            x[:, sl, :].transpose(1, 2, 0).reshape(K, B)).astype(np.float32)
        in_maps.append({"xT": xT, "xTf": xTf, "w1": w1, "w2": w2, "blk": blk})
    return in_maps


_NC_CACHE = {}


def _get_nc():
    if "nc" not in _NC_CACHE:
        _NC_CACHE["nc"] = build_kernel()
    return _NC_CACHE["nc"]


def _postprocess(v: np.ndarray) -> np.ndarray:
    out = v.reshape(CO, C, B).transpose(1, 2, 0)[:, :, None, None, :]
    return np.ascontiguousarray(out.astype(np.float32))


def kernel(x: np.ndarray, route_weights: np.ndarray) -> np.ndarray:
    nc = _get_nc()
    in_maps = _prep_inputs(np.asarray(x, np.float32),
                           np.asarray(route_weights, np.float32))
    res = bass_utils.run_bass_kernel_spmd(nc, in_maps,
                                          core_ids=list(range(NCORES)))
    return _postprocess(np.asarray(res.results[0]["vout"], np.float32))


def kernel_sim(x: np.ndarray, route_weights: np.ndarray) -> np.ndarray:
    """CoreSim (multi-core simulator) path for correctness debugging."""
    from concourse.bass_interp import MultiCoreSim
    nc = _get_nc()
    in_maps = _prep_inputs(np.asarray(x, np.float32),
                           np.asarray(route_weights, np.float32))
    sim = MultiCoreSim(nc, num_cores=NCORES)
    for i, core in sim.cores.items():
        for name, arr in in_maps[i].items():
            core.tensor(name)[:] = arr
    sim.simulate(check_with_hw=False)
    return _postprocess(np.asarray(sim.cores[0].tensor("vout"), np.float32))
